# revision 12
# baseline (speedup 1.0000x reference)
"""CrossAttention kernel for 8 Trainium2 NeuronCores.

Sharding: 8 cores = batch (2) x head-group (4, 8 heads each).
Per core: project q/k/v for its heads (k-outer loops, wide DMAs),
axial-RoPE q/k (pair-permuted layout, free-dim block rotation), PE
transposes to head-major layouts, attention with transposed logits
(ctx on partitions) and fused sumexp via a ones-column appended to v
(M=65 AV matmul), out-projection partial. Partials for the 4
head-groups of a batch are summed on the host (+ bias).

All matmuls run as float32r (full PE rate, ~1e-4 relative error).
"""

import sys

sys.path.insert(0, "/opt/trn_rl_repo")

import numpy as np

import concourse.bass as bass
import concourse.tile as tile
from concourse import bacc, mybir
from concourse.bass_utils import run_bass_kernel_spmd
from concourse.masks import make_identity

F32 = mybir.dt.float32
F32R = mybir.dt.float32r

DIM = 2048
CTX_DIM = 1024
N_HEADS = 32
HEAD_DIM = 64
B, N, CTXN = 2, 2048, 2048
ROPE_BASE = 10000.0

HPC = N_HEADS // 4          # heads per core = 8
HD_C = HPC * HEAD_DIM       # head dims per core = 512
NT = HD_C // 128            # 128-row tile groups (2 heads each) = 4
KC_X = DIM // 128            # contraction chunks for q proj = 16
KC_C = CTX_DIM // 128        # contraction chunks for kv proj = 8
NCH = N // 128               # n chunks = 16
CCH = CTXN // 128            # ctx chunks = 16
QC = 1024                    # attention q-column chunk
NQC = N // QC                # = 2
GRP_A = [3, 3, 3, 3, 3, 1]   # ctx-chunk group sizes (phase A)
GRP_B = [4, 4, 4, 4]         # n-chunk group sizes (phase B)

_BUILT = None

import contextlib
import os as _os

BF16 = _os.environ.get("KNL_BF16", "0") == "1"


@contextlib.contextmanager
def _nullctx():
    yield None



def _emit(nc, repeats=1):
    MDT = mybir.dt.bfloat16 if BF16 else F32R
    xp = nc.dram_tensor("xp", [DIM * N], MDT, kind="ExternalInput").ap()
    cp = nc.dram_tensor("cp", [CTX_DIM * CTXN], MDT, kind="ExternalInput").ap()
    wq = nc.dram_tensor("wq", [DIM, HD_C], MDT, kind="ExternalInput").ap()
    wk = nc.dram_tensor("wk", [CTX_DIM, HD_C], MDT, kind="ExternalInput").ap()
    wv = nc.dram_tensor("wv", [CTX_DIM, HD_C], MDT, kind="ExternalInput").ap()
    wo = nc.dram_tensor("wo", [HD_C, DIM], MDT, kind="ExternalInput").ap()
    cq = nc.dram_tensor("cq", [128, NCH, 32], F32, kind="ExternalInput").ap()
    sq = nc.dram_tensor("sq", [128, NCH, 32], F32, kind="ExternalInput").ap()
    ck = nc.dram_tensor("ck", [128, CCH, 32], F32, kind="ExternalInput").ap()
    sk = nc.dram_tensor("sk", [128, CCH, 32], F32, kind="ExternalInput").ap()
    out = nc.dram_tensor("out", [N, DIM], F32, kind="ExternalOutput").ap()

    with tile.TileContext(nc) as tc:
      with tc.For_i(0, repeats, 1) if repeats != 1 else _nullctx():
        with tc.tile_pool(name="const", bufs=1) as cpool, \
             tc.tile_pool(name="persist", bufs=1) as pp:
            tdt = mybir.dt.bfloat16 if BF16 else F32
            ident = cpool.tile([128, 128], tdt, tag="ident")
            make_identity(nc, ident[:])
            ones64f = cpool.tile([1, 64], F32, tag="ones64f")
            nc.gpsimd.memset(ones64f[:], 1.0)
            ones64 = cpool.tile([1, 64], F32R, tag="ones64")
            nc.vector.tensor_copy(ones64[:], ones64f[:])
            onescol = cpool.tile([128, 1], F32, tag="onescol")
            nc.gpsimd.memset(onescol[:], 1.0)

            # persistent tensors
            kT = [pp.tile([128, CTXN], MDT, tag=f"kT{t}", name=f"kT{t}")
                  for t in range(NT)]
            qT = [pp.tile([128, N], MDT, tag=f"qT{t}", name=f"qT{t}")
                  for t in range(NT)]
            vP = [pp.tile([128, HPC, 65], MDT, tag=f"vp{c}", name=f"vp{c}")
                  for c in range(CCH)]
            cq_t = pp.tile([128, NCH, 32], F32, tag="cq_t")
            sq_t = pp.tile([128, NCH, 32], F32, tag="sq_t")
            ck_t = pp.tile([128, CCH, 32], F32, tag="ck_t")
            sk_t = pp.tile([128, CCH, 32], F32, tag="sk_t")
            nc.sync.dma_start(cq_t[:], cq[:])
            nc.sync.dma_start(sq_t[:], sq[:])
            nc.sync.dma_start(ck_t[:], ck[:])
            nc.sync.dma_start(sk_t[:], sk[:])

            def rope(raw, rot, ctab, stab, tmp_pool):
                """raw/rot: [128, 512] f32 sbuf tiles; ctab/stab [128, 32] APs.
                Pair-permuted layout: per head, cols 0-31 = x1, 32-63 = x2.
                4 DVE ops: two full-width products, then cross sub/add."""
                rv = raw[:].rearrange("p (h two d) -> p h two d", two=2, d=32)
                r1 = rot[:].rearrange("p (h two d) -> p h two d", two=2, d=32)[:, :, 0, :]
                r2 = rot[:].rearrange("p (h two d) -> p h two d", two=2, d=32)[:, :, 1, :]

                def bc2(t):
                    return bass.AP(tensor=t.tensor, offset=t.offset,
                                   ap=[list(t.ap[0]), [0, HPC], [0, 2], [1, 32]])

                tcf = tmp_pool.tile([128, HPC, 2, 32], F32, tag="tcf")
                tsf = tmp_pool.tile([128, HPC, 2, 32], F32, tag="tsf")
                nc.vector.tensor_tensor(tcf[:], rv, bc2(ctab), mybir.AluOpType.mult)
                nc.vector.tensor_tensor(tsf[:], rv, bc2(stab), mybir.AluOpType.mult)
                nc.vector.tensor_sub(r1, tcf[:, :, 0, :], tsf[:, :, 1, :])
                nc.vector.tensor_add(r2, tsf[:, :, 0, :], tcf[:, :, 1, :])

            # ------------- Phases A+B: projections (k-outer, wide DMAs) -------
            with tc.tile_pool(name="wts", bufs=1) as wpool:
                wk_sb = [wpool.tile([128, HD_C], MDT, tag=f"wk{k}", name=f"wk{k}")
                         for k in range(KC_C)]
                wv_sb = [wpool.tile([128, HD_C], MDT, tag=f"wv{k}", name=f"wv{k}")
                         for k in range(KC_C)]
                wq_sb = [wpool.tile([128, HD_C], MDT, tag=f"wq{k}", name=f"wq{k}")
                         for k in range(KC_X)]
                for k in range(KC_C):
                    nc.sync.dma_start(wk_sb[k][:], wk[k * 128:(k + 1) * 128, :])
                    nc.sync.dma_start(wv_sb[k][:], wv[k * 128:(k + 1) * 128, :])

                ones_b = bass.AP(tensor=onescol[:].tensor, offset=onescol[:].offset,
                                 ap=[list(onescol[:].ap[0]), [0, HPC], [1, 1]])

                # ---- Phase A: k, v (groups of 3 ctx chunks) ----
                pend = []
                with tc.tile_pool(name="pa_sb", bufs=3) as sa, \
                     tc.tile_pool(name="pa_x", bufs=2) as xa, \
                     tc.tile_pool(name="pa_tmp", bufs=2) as tmpa, \
                     tc.tile_pool(name="pa_pk", bufs=3, space="PSUM") as ppk, \
                     tc.tile_pool(name="pa_pv", bufs=3, space="PSUM") as ppv, \
                     tc.tile_pool(name="pa_pt", bufs=2, space="PSUM") as ppt:

                    def flush_one_a():
                        if pend:
                            rot_t, cs_t = pend.pop(0)
                            for t in range(NT):
                                ptr = ppt.tile([128, 128], tdt, tag="ptr",
                                               name="ptra")
                                nc.tensor.transpose(
                                    ptr[:], rot_t[:, t * 128:(t + 1) * 128],
                                    ident[:])
                                nc.scalar.copy(kT[t][:, cs_t], ptr[:])

                    groups, _c0 = [], 0
                    for _g in GRP_A:
                        groups.append(list(range(_c0, _c0 + _g)))
                        _c0 += _g
                    blk_off = 0
                    for grp in groups:
                        gw = len(grp) * 128
                        pks = {c: ppk.tile([128, HD_C], F32, tag="pk", name="pk")
                               for c in grp}
                        pvs = {c: ppv.tile([128, HD_C], F32, tag="pv", name="pv")
                               for c in grp}
                        for k in range(KC_C):
                            xk = xa.tile([128, gw], MDT, tag="xk", name="xka")
                            src = bass.AP(tensor=cp.tensor, offset=blk_off,
                                          ap=[[gw, 128], [1, gw]])
                            nc.sync.dma_start(xk[:], src)
                            blk_off += 128 * gw
                            for j, c in enumerate(grp):
                                js = slice(j * 128, (j + 1) * 128)
                                nc.tensor.matmul(pks[c][:], xk[:, js], wk_sb[k][:],
                                                 start=(k == 0),
                                                 stop=(k == KC_C - 1))
                                nc.tensor.matmul(pvs[c][:], xk[:, js], wv_sb[k][:],
                                                 start=(k == 0),
                                                 stop=(k == KC_C - 1))
                            flush_one_a()
                        # drain PSUM k first (unblocks next group's matmuls),
                        # then v (DVE), then rope
                        kraws = {}
                        for c in grp:
                            kraw = sa.tile([128, HD_C], F32, tag="kraw")
                            nc.scalar.copy(kraw[:], pks[c][:])
                            kraws[c] = kraw
                        for c in grp:
                            nc.vector.tensor_copy(vP[c][:, :, 64:65], ones_b)
                            nc.vector.tensor_copy(
                                vP[c][:, :, 0:64],
                                pvs[c][:].rearrange("p (h d) -> p h d", d=64))
                        for c in grp:
                            cs = slice(c * 128, (c + 1) * 128)
                            rot = sa.tile([128, HD_C], tdt, tag="rot")
                            rope(kraws[c], rot, ck_t[:, c, :], sk_t[:, c, :], tmpa)
                            pend.append((rot, cs))
                    while pend:
                        flush_one_a()

                for k in range(KC_X):
                    nc.sync.dma_start(wq_sb[k][:], wq[k * 128:(k + 1) * 128, :])
                # ---- Phase B: q (groups of 6 n chunks) ----
                pend = []
                with tc.tile_pool(name="pb_sb", bufs=3) as sb_, \
                     tc.tile_pool(name="pb_x", bufs=2) as xb, \
                     tc.tile_pool(name="pb_tmp", bufs=2) as tmpb, \
                     tc.tile_pool(name="pb_pq", bufs=4, space="PSUM") as ppq, \
                     tc.tile_pool(name="pb_pt", bufs=2, space="PSUM") as pptb:

                    def flush_one_b():
                        if pend:
                            rot_t, ns_t = pend.pop(0)
                            for t in range(NT):
                                ptr = pptb.tile([128, 128], tdt, tag="ptr",
                                                name="ptrb")
                                nc.tensor.transpose(
                                    ptr[:], rot_t[:, t * 128:(t + 1) * 128],
                                    ident[:])
                                nc.scalar.copy(qT[t][:, ns_t], ptr[:])

                    groups, _c0 = [], 0
                    for _g in GRP_B:
                        groups.append(list(range(_c0, _c0 + _g)))
                        _c0 += _g
                    blk_off = 0
                    for grp in groups:
                        gw = len(grp) * 128
                        pqs = {c: ppq.tile([128, HD_C], F32, tag="pq", name="pq")
                               for c in grp}
                        for k in range(KC_X):
                            xk = xb.tile([128, gw], MDT, tag="xk", name="xkb")
                            src = bass.AP(tensor=xp.tensor, offset=blk_off,
                                          ap=[[gw, 128], [1, gw]])
                            nc.sync.dma_start(xk[:], src)
                            blk_off += 128 * gw
                            for j, c in enumerate(grp):
                                js = slice(j * 128, (j + 1) * 128)
                                nc.tensor.matmul(pqs[c][:], xk[:, js], wq_sb[k][:],
                                                 start=(k == 0),
                                                 stop=(k == KC_X - 1))
                            flush_one_b()
                        qraws = {}
                        for c in grp:
                            qraw = sb_.tile([128, HD_C], F32, tag="qraw")
                            nc.scalar.copy(qraw[:], pqs[c][:])
                            qraws[c] = qraw
                        for c in grp:
                            ns = slice(c * 128, (c + 1) * 128)
                            rot = sb_.tile([128, HD_C], tdt, tag="rot")
                            rope(qraws[c], rot, cq_t[:, c, :], sq_t[:, c, :], tmpb)
                            pend.append((rot, ns))
                    while pend:
                        flush_one_b()

            # ------------- Phase C: attention + out projection -------------
            with tc.tile_pool(name="pc_sb", bufs=3) as sc_, \
                 tc.tile_pool(name="pc_w", bufs=1) as wc, \
                 tc.tile_pool(name="pc_apack", bufs=NT) as ap_, \
                 tc.tile_pool(name="pc_lg", bufs=2, space="PSUM") as plg, \
                 tc.tile_pool(name="pc_av", bufs=1, space="PSUM") as pav, \
                 tc.tile_pool(name="pc_bc", bufs=1, space="PSUM") as pbc, \
                 tc.tile_pool(name="pc_op", bufs=1, space="PSUM") as pop:
                wout_sb = [wc.tile([128, DIM], MDT, tag=f"wo{t}", name=f"wo{t}")
                           for t in range(NT)]
                for t in range(NT):
                    nc.sync.dma_start(wout_sb[t][:], wo[t * 128:(t + 1) * 128, :])
                pend_out = []

                def emit_out_unit():
                    # one out-projection unit of the previous q chunk,
                    # used as PE filler inside ACT-bound attention loops
                    if not pend_out:
                        return
                    qc_p, ap_p, m, oc = pend_out.pop(0)
                    ms = slice(m * 128, (m + 1) * 128)
                    ocs = slice(oc * 512, (oc + 1) * 512)
                    po = pop.tile([128, 512], F32, tag="po", name="po")
                    for tt in range(NT):
                        nc.tensor.matmul(po[:], ap_p[tt][:, ms],
                                         wout_sb[tt][:, ocs],
                                         start=(tt == 0), stop=(tt == NT - 1))
                    so = sc_.tile([128, 512], F32, tag="so", bufs=2)
                    nc.vector.tensor_copy(so[:], po[:])
                    nc.sync.dma_start(
                        out[qc_p * QC + m * 128:qc_p * QC + (m + 1) * 128,
                            ocs], so[:])

                for qc in range(NQC):
                    apack = [ap_.tile([128, QC], MDT, tag="apack", name="apack",
                                      bufs=2 * NT)
                             for _ in range(NT)]
                    pend_norm = []

                    def drain_av(av_p, t_p, hs_p, qs_p):
                        # evacuate the AV PSUM bank eagerly so the next head's
                        # accumulation can start without waiting on normalize
                        araw = sc_.tile([64, 512], F32, tag="araw", name="araw", bufs=3)
                        nc.vector.tensor_copy(araw[:], av_p[0:64, :])
                        sume = sc_.tile([1, 512], F32, tag="sume", name="sume", bufs=3)
                        nc.vector.tensor_copy(sume[:], av_p[64:65, :])
                        pend_norm.append((araw, sume, t_p, hs_p, qs_p))

                    def flush_norm():
                        if not pend_norm:
                            return
                        araw, sume, t_p, hs_p, qs_p = pend_norm.pop(0)
                        recf = sc_.tile([1, 512], F32, tag="recf", name="recf", bufs=2)
                        nc.vector.reciprocal_approx_fast(recf[:], sume[:])
                        rec_t = sc_.tile([1, 512], F32R, tag="rec", name="rec", bufs=2)
                        nc.vector.tensor_copy(rec_t[:], recf[:])
                        bc_ = pbc.tile([64, 512], F32, tag="bc", name="bc")
                        nc.tensor.matmul(bc_[:], ones64[:], rec_t[:],
                                         start=True, stop=True)
                        nc.vector.tensor_mul(apack[t_p][hs_p, qs_p],
                                             araw[:], bc_[:])

                    # heads processed in pairs: both heads of tile t issue
                    # adjacent logits matmuls on array row groups 0-63/64-127
                    # (auto tile_position) -> concurrent, half the PE time.
                    # lg cols 0:512 = even head, 512:1024 = odd head, same
                    # 512-wide q block; both exp'd in one ACT op.
                    for qb2 in range(QC // 512):
                        qs = slice(qb2 * 512, (qb2 + 1) * 512)
                        uq = slice(qc * QC + qb2 * 512, qc * QC + (qb2 + 1) * 512)
                        for t in range(NT):
                            he, ho = 2 * t, 2 * t + 1
                            av_e = pav.tile([65, 512], F32, tag="av", name="av",
                                            bufs=2)
                            av_o = pav.tile([65, 512], F32, tag="av", name="av",
                                            bufs=2)

                            def emit_lg(c):
                                lg = plg.tile([128, QC], F32, tag="lg", name="lg")
                                cs = slice(c * 128, (c + 1) * 128)
                                nc.tensor.matmul(lg[:, 0:512], kT[t][0:64, cs],
                                                 qT[t][0:64, uq],
                                                 start=True, stop=True)
                                nc.tensor.matmul(lg[:, 512:1024],
                                                 kT[t][64:128, cs],
                                                 qT[t][64:128, uq],
                                                 start=True, stop=True)
                                ex = sc_.tile([128, QC], MDT, tag="ex", name="ex", bufs=2)
                                nc.scalar.activation(ex[:], lg[:],
                                                     mybir.ActivationFunctionType.Exp)
                                return ex

                            def emit_av(c, ex):
                                nc.tensor.matmul(av_e[:], vP[c][:, he, :],
                                                 ex[:, 0:512],
                                                 start=(c == 0),
                                                 stop=(c == CCH - 1))
                                nc.tensor.matmul(av_o[:], vP[c][:, ho, :],
                                                 ex[:, 512:1024],
                                                 start=(c == 0),
                                                 stop=(c == CCH - 1))

                            # software pipeline: lg(c+1) before av(c) keeps PE
                            # busy while ACT computes exp(c)
                            prev_ex = emit_lg(0)
                            for c in range(1, CCH):
                                ex = emit_lg(c)
                                emit_av(c - 1, prev_ex)
                                if c in (2, 4):
                                    flush_norm()  # previous pair's normalize
                                elif c in (6, 9, 12, 15):
                                    emit_out_unit()  # prev qc out-proj filler
                                prev_ex = ex
                            emit_av(CCH - 1, prev_ex)
                            drain_av(av_e, t, slice(0, 64), qs)
                            drain_av(av_o, t, slice(64, 128), qs)
                    while pend_norm:
                        flush_norm()
                    # queue this q chunk's out projection; it runs as filler
                    # inside the next q chunk's attention (tail-drained below)
                    for m in range(QC // 128):
                        for oc in range(DIM // 512):
                            pend_out.append((qc, apack, m, oc))
                while pend_out:
                    emit_out_unit()
    nc.compile()
    return nc


def _build():
    global _BUILT
    if _BUILT is None:
        nc = bacc.Bacc("TRN2", target_bir_lowering=False, debug=False)
        _BUILT = _emit(nc)
    return _BUILT


def _pair_perm():
    # within a head: x1 dims (even pair members) then x2 dims
    x1 = [p * 32 + 2 * i for p in range(2) for i in range(16)]
    x2 = [p * 32 + 2 * i + 1 for p in range(2) for i in range(16)]
    return np.array(x1 + x2, dtype=np.int64)


def _rope_tables(pos, scale, nch):
    # pos: (n, 2) -> packed cos/sin tables (128, nch, 32), col j=(p=j//16, i=j%16)
    freqs = ROPE_BASE ** (-np.arange(16, dtype=np.float32) / 16.0)
    ang = pos[:, :, None].astype(np.float32) * freqs[None, None, :]
    ang = ang.reshape(pos.shape[0], 32)
    c = (np.cos(ang) * scale).astype(np.float32)
    s = (np.sin(ang) * scale).astype(np.float32)
    # pack rows: row n = c*128 + p  ->  [p, c, 32]
    c = np.ascontiguousarray(c.reshape(nch, 128, 32).transpose(1, 0, 2))
    s = np.ascontiguousarray(s.reshape(nch, 128, 32).transpose(1, 0, 2))
    return c, s


def kernel(x, ctx, pos_map, ctx_pos_map, Wq, Wkv, Wout, bout):
    x = np.asarray(x, dtype=np.float32)
    ctx = np.asarray(ctx, dtype=np.float32)
    pos_map = np.asarray(pos_map, dtype=np.float32)
    ctx_pos_map = np.asarray(ctx_pos_map, dtype=np.float32)
    Wq = np.asarray(Wq, dtype=np.float32)
    Wkv = np.asarray(Wkv, dtype=np.float32)
    Wout = np.asarray(Wout, dtype=np.float32)
    bout = np.asarray(bout, dtype=np.float32)

    nc = _build()

    perm = _pair_perm()
    scale = 1.0 / np.sqrt(np.float32(HEAD_DIM))

    mdt_np = np.float32
    if BF16:
        import ml_dtypes
        mdt_np = ml_dtypes.bfloat16

    def _blocks(mat_t, grp_sizes, kc):
        # mat_t: [d, n]; blocks in group-outer, k-inner order, contiguous
        parts, c0 = [], 0
        for g in grp_sizes:
            gw = g * 128
            for k in range(kc):
                parts.append(mat_t[k * 128:(k + 1) * 128, c0:c0 + gw].ravel())
            c0 += gw
        return np.ascontiguousarray(np.concatenate(parts)).astype(mdt_np)

    xT = [_blocks(x[b].T, GRP_B, KC_X) for b in range(B)]
    ctxT = [_blocks(ctx[b].T, GRP_A, KC_C) for b in range(B)]
    qtabs = [_rope_tables(pos_map[b], scale, NCH) for b in range(B)]
    ktabs = [_rope_tables(ctx_pos_map[b], 1.0, CCH) for b in range(B)]

    Wk_full, Wv_full = Wkv[:, :DIM], Wkv[:, DIM:]

    in_maps = []
    for core in range(8):
        b, hg = core // 4, core % 4
        hcols = np.arange(hg * HD_C, (hg + 1) * HD_C)
        pcols = np.concatenate([hg * HD_C + h * 64 + perm for h in range(HPC)])
        in_maps.append({
            "xp": xT[b],
            "cp": ctxT[b],
            "wq": np.ascontiguousarray(Wq[:, pcols]).astype(mdt_np),
            "wk": np.ascontiguousarray(Wk_full[:, pcols]).astype(mdt_np),
            "wv": np.ascontiguousarray(Wv_full[:, hcols]).astype(mdt_np),
            "wo": np.ascontiguousarray(Wout[hcols, :]).astype(mdt_np),
            "cq": qtabs[b][0], "sq": qtabs[b][1],
            "ck": ktabs[b][0], "sk": ktabs[b][1],
        })

    res = run_bass_kernel_spmd(nc, in_maps, list(range(8)))
    kernel.last_results = res

    out = np.empty((B, N, DIM), dtype=np.float32)
    for b in range(B):
        acc = res.results[4 * b]["out"].astype(np.float32).copy()
        for hg in range(1, 4):
            acc += res.results[4 * b + hg]["out"]
        out[b] = acc + bout[None, :]
    return out

